# revision 30
# baseline (speedup 1.0000x reference)
"""ChebNet (K=4, 2 ChebConv layers + relu + log_softmax) on 8 trn2 NeuronCores.

Strategy (graph/data parallel, dense-ified SpMM on the TensorEngine):
  - prop(h) = A @ h with A = -diag(dis) @ Cnt @ diag(dis); Cnt dense-ified
    in fp8 (exact small ints). Nodes sharded 8 ways by destination; each
    core keeps its Cnt^T shard [src=N_pad, dst=DLOC] SBUF-resident and
    computes its 1/8 of every propagation as chained PE matmuls:
    lhsT = bf16 tiles of dis*T_{k-1} (all-gathered "g"), rhs = fp8 Cnt^T
    columns, fp32 PSUM accumulation.
  - Between steps the per-core [DLOC, F] bf16 shards are exchanged with a
    chunked 8-core AllGather, overlapped with the next chunk's chain.

Changes vs the 436µs baseline (trace-driven):
  - Per-order weight matmuls run in bf16 on the scaled stream g_k = dis*T_k
    (already materialized for staging) and accumulate k=0..3 in a persistent
    PSUM bank per chunk; the layer end multiplies by dinv = sqrt(deg) to
    unscale. Replaces ~28µs of fp32 PE work with ~8µs bf16.
  - The Cnt input is host-permuted into exact chain consumption order so the
    startup load paces the first chain, and the final all-padding src tile
    (g rows identically zero) is skipped in every chain.
  - Queue discipline: send path (stage transposes' copies + ag_src DMA) on
    sync, receive path (AG->SBUF g-loads) deferred to the consuming chain's
    start on sync+scalar tails, AG triggers alone on gpsimd — so no
    long-blocking g-load ever head-blocks a stage-out or an AG trigger.
  - Warm-filler matmuls are dependency-free (identity rhs) and sit at the
    exchange boundaries plus the two mid-chain consumption points of the
    first post-exchange chain, keeping the PE HAM clock at 8/8 across the
    short AllGather waits.

Not viable (measured): XBAR DMA-transpose staging (HW cost ~1.2µs per
128x128 tile, 10x the cost model; serialized the send path), a tiny
comm-warming AllGather (queues behind the ~50µs first-collective barrier
and delays exchange 1), bulk strided g-loads (single DMA engine, 2x
slower than 8 parallel per-core loads), pp PSUM bufs=3 (+10µs).
"""

import sys

sys.path.insert(0, "/opt/trn_rl_repo")

import numpy as np
import ml_dtypes

import concourse.bacc as bacc
import concourse.mybir as mybir
import concourse.tile as tile
from concourse.bass_utils import run_bass_kernel_spmd
from concourse.masks import make_identity

F32 = mybir.dt.float32
BF16 = mybir.dt.bfloat16
F8E4 = mybir.dt.float8e4

NCORES = 8
P = 128

N = 10000
F_IN = 128
HID = 128
C_OUT = 16
K_ORD = 4


class Geom:
    """Problem geometry. tiles_per_core src-tiles of 128 nodes per core."""

    def __init__(self, n_nodes, tiles_per_core, f_in=F_IN, hid=HID, c_out=C_OUT,
                 k_ord=K_ORD):
        self.n = n_nodes
        self.tpc = tiles_per_core          # src tiles per core (DLOC/128)
        self.dloc = tiles_per_core * P     # nodes per core (padded)
        self.npad = self.dloc * NCORES     # padded node count
        self.nt = self.npad // P           # total src tiles
        self.f = f_in
        self.hid = hid
        self.c = c_out
        self.k = k_ord
        assert self.npad >= n_nodes
        assert f_in == P and hid == P
        # psum chunking of the dloc free dim (max 512 fp32 per bank)
        self.chunks = []
        off = 0
        while off < self.dloc:
            sz = min(512, self.dloc - off)
            self.chunks.append((off, sz))
            off += sz
        # per-chunk local tile ranges
        self.ctiles = [(off // P, (off + sz) // P) for off, sz in self.chunks]
        # chain consumption order of the 80 src tiles: all chunk-0 g tiles
        # (earliest AllGather) first, then chunk 1, then chunk 2.
        self.tile_order = [(gci, j, t)
                           for gci, (t0, t1) in enumerate(self.ctiles)
                           for j in range(NCORES)
                           for t in range(t0, t1)]
        # the last consumed src tile is pure padding (dis=0 -> g rows are
        # all zero) when its first node is >= n: skip its matmuls.
        if (self.nt - 1) * P >= n_nodes:
            self.tile_order = self.tile_order[:-1]
        # A is loaded in slabs of 8 consecutive consumption positions
        self.slab = 8
        assert self.nt % self.slab == 0
        self.n_slab = self.nt // self.slab


FULL = Geom(N, 10)  # 1280 nodes/core, npad=10240, 80 src tiles

# PE-warming filler matmuls placed at the known stall windows (start of a
# chain whose gathered g may not have landed yet) so the HAM clock gate
# stays at 8/8 across short waits.
N_WARM_START = 30
N_WARM_STEP = 18
N_WARM_FIRST = 250


def build_nc(g: Geom):
    nc = bacc.Bacc("TRN2", target_bir_lowering=False, debug=False,
                   num_devices=NCORES)

    # ---- kernel I/O ----------------------------------------------------
    # fp8 Cnt^T shard, consumption-order slabs per psum chunk:
    # [n_slab, 128, slab, chunk_sz]
    a_in = [nc.dram_tensor(f"a_in_c{ci}", [g.n_slab, P, g.slab, sz], F8E4,
                           kind="ExternalInput")
            for ci, (off, sz) in enumerate(g.chunks)]
    # initial g = dis * x, tiled [128, nt, f]
    g0_in = nc.dram_tensor("g0_in", [P, g.nt, g.f], BF16, kind="ExternalInput")
    # local x^T shard fp32 (T0 in transposed layout, for the recursion)
    xt_in = nc.dram_tensor("xt_in", [P, g.dloc], F32, kind="ExternalInput")
    # local dis*x^T bf16 (rhs of the k=0 weight term, layer 1)
    g0f_in = nc.dram_tensor("g0f_in", [P, g.dloc], BF16, kind="ExternalInput")
    # broadcast +dis rows and dinv=sqrt(deg) rows for the local shard
    disp_in = nc.dram_tensor("disp_in", [P, g.dloc], F32, kind="ExternalInput")
    dinv_in = nc.dram_tensor("dinv_in", [P, g.dloc], F32, kind="ExternalInput")
    w1_in = nc.dram_tensor("w1_in", [P, g.k, g.hid], BF16, kind="ExternalInput")
    w2_in = nc.dram_tensor("w2_in", [P, g.k, g.c], BF16, kind="ExternalInput")
    # bias columns: col 0 = b1 (hid rows), col 1 = b2 (c rows)
    bb_in = nc.dram_tensor("bb_in", [P, 2], F32, kind="ExternalInput")

    out_dram = nc.dram_tensor("out", [g.dloc, g.c], F32, kind="ExternalOutput")

    n_ag = 5  # exchanges: L1 T1, L1 T2, h, L2 T1, L2 T2

    with tile.TileContext(nc) as tc:
        with (
            tc.tile_pool(name="pers", bufs=1) as pers,
            tc.tile_pool(name="work", bufs=1) as work,
            tc.tile_pool(name="psum", bufs=1, space="PSUM") as psp,
            tc.tile_pool(name="dram", bufs=1, space="DRAM") as drp,
        ):
            # ---- persistent SBUF ---------------------------------------
            a_sb = [[pers.tile([P, g.slab, sz], F8E4, tag=f"a{ci}_{s}",
                               name=f"a{ci}_{s}")
                     for s in range(g.n_slab)]
                    for ci, (off, sz) in enumerate(g.chunks)]
            gbufC = [[pers.tile([P, NCORES, t1 - t0, g.f], BF16,
                                tag=f"g{b}_{ci}", name=f"g{b}_{ci}")
                      for ci, (t0, t1) in enumerate(g.ctiles)]
                     for b in range(2)]
            t_sb = [pers.tile([P, g.dloc], F32, tag=f"t{i}", name=f"t{i}")
                    for i in range(3)]
            disp = pers.tile([P, g.dloc], F32, name="disp")
            dinv = pers.tile([P, g.dloc], F32, name="dinv")
            gcast = pers.tile([P, g.dloc], BF16, name="gcast")
            g0f = pers.tile([P, g.dloc], BF16, name="g0f")
            hgf = pers.tile([P, g.dloc], BF16, name="hgf")
            tstage = pers.tile([P, g.tpc, g.f], BF16, name="tstage")
            w1_sb = pers.tile([P, g.k, g.hid], BF16, name="w1_sb")
            w2_sb = pers.tile([P, g.k, g.c], BF16, name="w2_sb")
            bb_sb = pers.tile([P, 2], F32, name="bb_sb")
            zf = pers.tile([g.c, g.dloc], F32, name="zf")
            idf32 = pers.tile([P, P], F32, name="idf32")
            idbf = pers.tile([P, P], BF16, name="idbf")

            # ---- DRAM bounce buffers for the collectives ---------------
            # payload row = node-in-tile (after the XBAR transpose),
            # col = (tile, feature).
            ag_srcC = [[drp.tile([P, sz], BF16, name=f"ag_src{i}_{ci}")
                        for ci, (off, sz) in enumerate(g.chunks)]
                       for i in range(n_ag)]
            ag_dstC = [[drp.tile([NCORES * P, sz], BF16,
                                 addr_space="Shared", name=f"ag_dst{i}_{ci}")
                        for ci, (off, sz) in enumerate(g.chunks)]
                       for i in range(n_ag)]
            # tiny collective to absorb comm-init + first rendezvous while
            # the inputs are still loading
            make_identity(nc, idf32[:])
            make_identity(nc, idbf[:])

            def warm(n_mm):
                """Open the PE HAM clock gate with dummy matmuls."""
                for _ in range(n_mm):
                    wp = psp.tile([P, 256], F32, space="PSUM", tag="warm",
                                  name="wp")
                    nc.tensor.matmul(wp[:, :128], lhsT=idbf[:],
                                     rhs=idbf[:], start=True, stop=True,
                                     skip_group_check=True)

            warm(N_WARM_START)

            # ---- input loads, in chain consumption order ---------------
            dges = [nc.sync, nc.scalar, nc.gpsimd]
            ld = 0

            def dma_in(dst, src):
                nonlocal ld
                dges[ld % 3].dma_start(dst, src)
                ld += 1

            dma_in(g0f[:], g0f_in[:])
            dma_in(disp[:], disp_in[:])
            dma_in(w1_sb[:], w1_in[:])
            # chunk ci of the first chain needs: its 8 g0 core-chunks and its
            # 10 A slabs, interleaved so consumption never outruns the DMA.
            for ci, (t0, t1) in enumerate(g.ctiles):
                for s in range(max(g.n_slab, NCORES)):
                    if s < NCORES:
                        j = s
                        dma_in(gbufC[0][ci][:, j, :, :],
                               g0_in[:, j * g.tpc + t0:j * g.tpc + t1, :])
                    if s < g.n_slab:
                        dma_in(a_sb[ci][s][:], a_in[ci][s])
            dma_in(t_sb[0][:], xt_in[:])
            dma_in(dinv[:], dinv_in[:])
            dma_in(w2_sb[:], w2_in[:])
            dma_in(bb_sb[:], bb_in[:])

            ag_idx = 0
            cur = 0  # g-buffer ping-pong index; gbuf[0] holds g(x)

            def stage_chunk(idx, ci, gf_src, off, sz):
                """Transpose gf chunk to node-major (PE), DMA to ag_src.
                The send path DMA lives on the sync queue only, so it can
                never head-block behind a receive-side g-load."""
                t0 = off // P
                for t in range((off) // P, (off + sz) // P):
                    tpb = psp.tile([P, P], BF16, space="PSUM", tag="tpb",
                                   name="tpb", bufs=2)
                    nc.tensor.transpose(out=tpb[:],
                                        in_=gf_src[:, t * P:(t + 1) * P],
                                        identity=idbf[:])
                    nc.vector.tensor_copy(tstage[:, t, :], tpb[:])
                nc.sync.dma_start(ag_srcC[idx][ci][:],
                                  tstage[:, t0:t0 + sz // P, :])

            gload_queue = []

            def allgather_chunk(idx, ci, b_next):
                nc.gpsimd.collective_compute(
                    "AllGather",
                    mybir.AluOpType.bypass,
                    replica_groups=[list(range(NCORES))],
                    ins=[ag_srcC[idx][ci][:]],
                    outs=[ag_dstC[idx][ci][:]],
                )
                gload_queue.append((idx, ci, b_next))

            def flush_gloads():
                """Emit the previous exchange's AG->SBUF loads. Deferred to
                the start of the consuming chain so they queue at the TAIL
                of the sync/scalar queues: nothing time-critical (stage-outs,
                AG triggers) ever head-blocks behind their AG waits."""
                gq = 0
                for idx, ci, b_next in gload_queue:
                    for j in range(NCORES):
                        eng = nc.sync if gq % 2 == 0 else nc.scalar
                        gq += 1
                        eng.dma_start(
                            gbufC[b_next][ci][:, j, :, :],
                            ag_dstC[idx][ci][j * P:(j + 1) * P, :]
                            .rearrange("p (t f) -> p t f", f=g.f),
                        )
                gload_queue.clear()

            z_all = work.tile([P, g.tpc, g.c], F32, name="z_all")
            m_all = work.tile([P, g.tpc, 1], F32, name="m_all")
            e_all = work.tile([P, g.tpc, g.c], F32, name="e_all")
            s_all = work.tile([P, g.tpc, 1], F32, name="s_all")

            def softmax_chunk(off, sz):
                t0, t1 = off // P, (off + sz) // P
                za = z_all[:, t0:t1, :]
                ma = m_all[:, t0:t1, :]
                ea = e_all[:, t0:t1, :]
                sa = s_all[:, t0:t1, :]
                shp = [P, t1 - t0, g.c]
                nc.vector.tensor_reduce(out=ma[:, :, 0], in_=za,
                                        axis=mybir.AxisListType.X,
                                        op=mybir.AluOpType.max)
                nc.vector.tensor_tensor(out=ea, in0=za,
                                        in1=ma.to_broadcast(shp),
                                        op=mybir.AluOpType.subtract)
                nc.scalar.activation(ea, ea,
                                     mybir.ActivationFunctionType.Exp)
                nc.vector.tensor_reduce(out=sa[:, :, 0], in_=ea,
                                        axis=mybir.AxisListType.X,
                                        op=mybir.AluOpType.add)
                nc.scalar.activation(sa, sa,
                                     mybir.ActivationFunctionType.Ln)
                nc.vector.tensor_add(sa, sa, ma)
                # out = z - (m + ln s), written per chunk
                nc.vector.tensor_tensor(out=ea, in0=za,
                                        in1=sa.to_broadcast(shp),
                                        op=mybir.AluOpType.subtract)
                nc.sync.dma_start(
                    out_dram.ap().rearrange("(t p) c -> p t c", p=P)
                    [:, t0:t1, :],
                    ea,
                )

            # pending PE work (weight matmuls, layer-end finalization) that
            # must trail the DVE recursion by a few chain matmuls so the
            # in-order PE never stalls on DVE results.
            pending = []

            def fire_pending():
                # High priority: the deferred W-matmuls / staging / layer-end
                # work feeds the exchange critical path; without this the
                # tile scheduler (whose internal cost model mis-estimates
                # collective and DMA-transpose latencies) slots it a full
                # chunk-chain later than emission order.
                with tc.high_priority():
                    for fn in pending:
                        fn()
                pending.clear()

            # ---- the two ChebConv layers -------------------------------
            for layer in range(2):
                w_sb = w1_sb if layer == 0 else w2_sb
                cdim = g.hid if layer == 0 else g.c
                gf0 = g0f if layer == 0 else hgf
                # persistent per-chunk PSUM accumulators for the W terms
                wt = [psp.tile([P, sz], F32, space="PSUM", tag=f"wt{ci}",
                               name=f"wt{ci}")
                      for ci, (off, sz) in enumerate(g.chunks)]

                # k=0 weight term (deferred into the T1 chain below)
                for ci, (off, sz) in enumerate(g.chunks):
                    def w0_term(ci=ci, off=off, sz=sz, wt=wt, w_sb=w_sb,
                                cdim=cdim, gf0=gf0):
                        nc.tensor.matmul(
                            wt[ci][:cdim, :sz],
                            lhsT=w_sb[:, 0, :cdim],
                            rhs=gf0[:, off:off + sz],
                            start=True, stop=False,
                        )
                    pending.append(w0_term)

                for k in range(1, g.k):
                    tk = t_sb[k % 3]
                    tk2 = t_sb[(k - 2) % 3] if k >= 2 else None
                    do_stage = k < g.k - 1  # T3 itself is never exchanged
                    do_ag = do_stage or layer == 0
                    flush_gloads()

                    for ci, (off, sz) in enumerate(g.chunks):
                        pp = psp.tile([P, 512], F32, space="PSUM", tag="pp",
                                      name="pp", bufs=2)
                        n_mm = len(g.tile_order)
                        # first chunk-chain after an exchange: its remote
                        # g-chunks can land a few µs late; short filler
                        # bursts right before the consumption points keep
                        # the HAM clock warm across those waits.
                        fresh = ci == 0 and not (layer == 0 and k == 1)
                        for n_i, (gci, j, t) in enumerate(g.tile_order):
                            lhs = gbufC[cur][gci][:, j, t - g.ctiles[gci][0], :]
                            nc.tensor.matmul(
                                pp[:, :sz],
                                lhsT=lhs,
                                rhs=a_sb[ci][n_i // g.slab][:, n_i % g.slab, :],
                                start=(n_i == 0),
                                stop=(n_i == n_mm - 1),
                            )
                            if n_i == 15:
                                fire_pending()
                            if fresh and n_mm > 64 and n_i in (31, 63):
                                warm(6 if n_i == 31 else 4)
                        # Chebyshev recursion (fp32, on DVE). High priority:
                        # this is the head of the exchange critical path.
                        with tc.high_priority():
                            nc.vector.scalar_tensor_tensor(
                                out=tk[:, off:off + sz],
                                in0=pp[:, :sz],
                                scalar=-1.0 if k == 1 else -2.0,
                                in1=disp[:, off:off + sz],
                                op0=mybir.AluOpType.mult,
                                op1=mybir.AluOpType.mult)
                            if k >= 2:
                                nc.vector.tensor_sub(
                                    tk[:, off:off + sz],
                                    tk[:, off:off + sz],
                                    tk2[:, off:off + sz])
                            # g_k = dis * T_k (bf16): chain lhsT for the
                            # next prop, staged, and W-term rhs.
                            nc.vector.tensor_tensor(
                                out=gcast[:, off:off + sz],
                                in0=tk[:, off:off + sz],
                                in1=disp[:, off:off + sz],
                                op=mybir.AluOpType.mult)
                        def w_term(ci=ci, off=off, sz=sz, k=k, wt=wt,
                                   w_sb=w_sb, cdim=cdim,
                                   do_stage=do_stage):
                            nc.tensor.matmul(
                                wt[ci][:cdim, :sz],
                                lhsT=w_sb[:, k, :cdim],
                                rhs=gcast[:, off:off + sz],
                                start=False, stop=(k == g.k - 1),
                            )
                            if do_stage:
                                stage_chunk(ag_idx, ci, gcast, off, sz)
                                allgather_chunk(ag_idx, ci, 1 - cur)
                                if ci == len(g.chunks) - 1:
                                    warm(N_WARM_FIRST if ag_idx == 0
                                         else N_WARM_STEP)
                        pending.append(w_term)

                        if k == 3 and layer == 0:
                            # layer 1 end: h = relu(dinv*wt + b1) -> t_sb[0],
                            # hgf = dis*h, stage + exchange (all off-PE).
                            def l1_end(ci=ci, off=off, sz=sz, wt=wt):
                                nc.vector.tensor_tensor(
                                    out=t_sb[0][:, off:off + sz],
                                    in0=wt[ci][:, :sz],
                                    in1=dinv[:, off:off + sz],
                                    op=mybir.AluOpType.mult)
                                nc.scalar.activation(
                                    t_sb[0][:, off:off + sz],
                                    t_sb[0][:, off:off + sz],
                                    mybir.ActivationFunctionType.Relu,
                                    bias=bb_sb[:, 0:1], scale=1.0)
                                nc.vector.tensor_tensor(
                                    out=hgf[:, off:off + sz],
                                    in0=t_sb[0][:, off:off + sz],
                                    in1=disp[:, off:off + sz],
                                    op=mybir.AluOpType.mult)
                                stage_chunk(ag_idx, ci, hgf, off, sz)
                                allgather_chunk(ag_idx, ci, 1 - cur)
                                if ci == len(g.chunks) - 1:
                                    warm(N_WARM_STEP)
                            pending.append(l1_end)

                        if k == 3 and layer == 1:
                            def l2_end(ci=ci, off=off, sz=sz, wt=wt):
                                nc.vector.tensor_tensor(
                                    out=zf[:, off:off + sz],
                                    in0=wt[ci][:g.c, :sz],
                                    in1=dinv[:g.c, off:off + sz],
                                    op=mybir.AluOpType.mult)
                                nc.scalar.activation(
                                    zf[:, off:off + sz],
                                    zf[:, off:off + sz],
                                    mybir.ActivationFunctionType.Identity,
                                    bias=bb_sb[:g.c, 1:2], scale=1.0)
                                for t in range(off // P, (off + sz) // P):
                                    zp = psp.tile([P, g.c], F32, space="PSUM",
                                                  tag="tpb", name="zp", bufs=2)
                                    nc.tensor.transpose(
                                        out=zp[:],
                                        in_=zf[:, t * P:(t + 1) * P],
                                        identity=idf32[:g.c, :g.c])
                                    nc.vector.tensor_copy(
                                        z_all[:, t, :], zp[:])
                                softmax_chunk(off, sz)
                            pending.append(l2_end)

                    fire_pending()
                    if do_ag:
                        ag_idx += 1
                        cur = 1 - cur

    nc.compile()
    return nc


def host_prep(g: Geom, x, edge_index, W1, b1, W2, b2):
    """Build the per-core input maps (sharding + dense-ification)."""
    n = g.n
    src = np.asarray(edge_index[0], dtype=np.int64)
    dst = np.asarray(edge_index[1], dtype=np.int64)
    deg = np.bincount(src, minlength=n).astype(np.float64)
    dis = np.where(deg > 0, 1.0 / np.sqrt(np.maximum(deg, 1e-12)), 0.0)
    dnv = np.where(deg > 0, np.sqrt(deg), 0.0)

    # dense-ified edge-count matrix, transposed: cnt_t[s, d]
    cnt_t = np.zeros((g.npad, g.npad), dtype=np.float32)
    np.add.at(cnt_t, (src, dst), 1.0)

    dis_pad = np.zeros(g.npad, dtype=np.float32)
    dis_pad[:n] = dis.astype(np.float32)
    dnv_pad = np.zeros(g.npad, dtype=np.float32)
    dnv_pad[:n] = dnv.astype(np.float32)
    x_pad = np.zeros((g.npad, g.f), dtype=np.float32)
    x_pad[:n] = np.asarray(x, dtype=np.float32)

    g0 = dis_pad[:, None] * x_pad  # [npad, f]
    g0_tiles = (g0.reshape(g.nt, P, g.f).transpose(1, 0, 2)
                .astype(ml_dtypes.bfloat16))  # [128, nt, f]

    w1 = np.ascontiguousarray(
        np.asarray(W1, np.float32).transpose(1, 0, 2)).astype(
            ml_dtypes.bfloat16)  # [P, k, hid]
    w2 = np.ascontiguousarray(
        np.asarray(W2, np.float32).transpose(1, 0, 2)).astype(
            ml_dtypes.bfloat16)  # [P, k, c]
    bb = np.zeros((P, 2), np.float32)
    bb[:g.hid, 0] = np.asarray(b1, np.float32)
    bb[:g.c, 1] = np.asarray(b2, np.float32)

    # chain consumption order of src tiles (must match Geom.tile_order);
    # tiles skipped by the chain (pure padding) pack at the end.
    order = [j * g.tpc + t for (gci, j, t) in g.tile_order]
    perm = np.array(order + [gi for gi in range(g.nt)
                             if gi not in set(order)])

    in_maps = []
    for c in range(NCORES):
        lo, hi = c * g.dloc, (c + 1) * g.dloc
        a_c = (cnt_t[:, lo:hi].astype(ml_dtypes.float8_e4m3)
               .reshape(g.nt, P, g.dloc))[perm]   # [nt, P, dloc] in order
        a_c = (a_c.reshape(g.n_slab, g.slab, P, g.dloc)
               .transpose(0, 2, 1, 3))            # [n_slab, P, slab, dloc]
        im = {f"a_in_c{ci}":
              np.ascontiguousarray(a_c[:, :, :, off:off + sz])
              for ci, (off, sz) in enumerate(g.chunks)}
        xt = np.ascontiguousarray(x_pad[lo:hi].T)          # [128, dloc]
        g0f = np.ascontiguousarray(
            g0[lo:hi].T.astype(ml_dtypes.bfloat16))        # [128, dloc]
        d_loc = dis_pad[lo:hi]
        disp = np.ascontiguousarray(
            np.broadcast_to(d_loc[None, :], (P, g.dloc))).astype(np.float32)
        dinv = np.ascontiguousarray(
            np.broadcast_to(dnv_pad[lo:hi][None, :],
                            (P, g.dloc))).astype(np.float32)
        im.update({
            "g0_in": np.ascontiguousarray(g0_tiles),
            "xt_in": xt,
            "g0f_in": g0f,
            "disp_in": disp,
            "dinv_in": dinv,
            "w1_in": w1,
            "w2_in": w2,
            "bb_in": bb,
        })
        in_maps.append(im)
    return in_maps


_CACHED_NC = None


def _get_nc():
    global _CACHED_NC
    if _CACHED_NC is None:
        _CACHED_NC = build_nc(FULL)
    return _CACHED_NC


def _enable_ldw_opt():
    """The default axon compile flags pass --enable-ldw-opt=false, which
    serializes every LDWEIGHTS with its MATMUL (~+107ns per matmul). Our
    kernel is a long stream of ldweights+matmul pairs, so re-enable it."""
    try:
        from concourse.compiler_utils import (get_compiler_flags,
                                              set_compiler_flags)
        flags = get_compiler_flags()
        new = [f.replace("--enable-ldw-opt=false", "--enable-ldw-opt=true")
               for f in flags]
        if new != flags:
            set_compiler_flags(new)
    except Exception:
        pass


def kernel(x, edge_index, W1, b1, W2, b2, _profile=False):
    g = FULL
    _enable_ldw_opt()
    in_maps = host_prep(g, x, edge_index, W1, b1, W2, b2)
    nc = _get_nc()
    res = run_bass_kernel_spmd(nc, in_maps, list(range(NCORES)),
                               trace=_profile)
    out = np.concatenate([res.results[c]["out"] for c in range(NCORES)], 0)
    out = out[:g.n].astype(np.float32)
    if _profile:
        kernel.last_result = res
    return out


# revision 31
# speedup vs baseline: 1.0721x; 1.0721x over previous
"""ChebNet (K=4, 2 ChebConv layers + relu + log_softmax) on 8 trn2 NeuronCores.

Strategy (graph/data parallel, dense-ified SpMM on the TensorEngine):
  - prop(h) = A @ h with A = -diag(dis) @ Cnt @ diag(dis); Cnt dense-ified
    in fp8 (exact small ints). Nodes sharded 8 ways by destination; each
    core keeps its Cnt^T shard [src=N_pad, dst=DLOC] SBUF-resident and
    computes its 1/8 of every propagation as chained PE matmuls:
    lhsT = bf16 tiles of dis*T_{k-1} (all-gathered "g"), rhs = fp8 Cnt^T
    columns, fp32 PSUM accumulation.
  - Between steps the per-core [DLOC, F] bf16 shards are exchanged with a
    chunked 8-core AllGather, overlapped with the next chunk's chain.

Changes vs the 436µs baseline (trace-driven):
  - Per-order weight matmuls run in bf16 on the scaled stream g_k = dis*T_k
    (already materialized for staging) and accumulate k=0..3 in a persistent
    PSUM bank per chunk; the layer end multiplies by dinv = sqrt(deg) to
    unscale. Replaces ~28µs of fp32 PE work with ~8µs bf16.
  - The Cnt input is host-permuted into exact chain consumption order so the
    startup load paces the first chain, and the final all-padding src tile
    (g rows identically zero) is skipped in every chain.
  - Queue discipline: send path (stage transposes' copies + ag_src DMA) on
    sync, receive path (AG->SBUF g-loads) deferred to the consuming chain's
    start on sync+scalar tails, AG triggers alone on gpsimd — so no
    long-blocking g-load ever head-blocks a stage-out or an AG trigger.
  - Warm-filler matmuls are dependency-free (identity rhs) and sit at the
    exchange boundaries plus the two mid-chain consumption points of the
    first post-exchange chain, keeping the PE HAM clock at 8/8 across the
    short AllGather waits.

Not viable (measured): XBAR DMA-transpose staging (HW cost ~1.2µs per
128x128 tile, 10x the cost model; serialized the send path), a tiny
comm-warming AllGather (queues behind the ~50µs first-collective barrier
and delays exchange 1), bulk strided g-loads (single DMA engine, 2x
slower than 8 parallel per-core loads), pp PSUM bufs=3 (+10µs).
"""

import sys

sys.path.insert(0, "/opt/trn_rl_repo")

import numpy as np
import ml_dtypes

import concourse.bacc as bacc
import concourse.mybir as mybir
import concourse.tile as tile
from concourse.bass_utils import run_bass_kernel_spmd
from concourse.masks import make_identity

F32 = mybir.dt.float32
BF16 = mybir.dt.bfloat16
F8E4 = mybir.dt.float8e4

NCORES = 8
P = 128

N = 10000
F_IN = 128
HID = 128
C_OUT = 16
K_ORD = 4


class Geom:
    """Problem geometry. tiles_per_core src-tiles of 128 nodes per core."""

    def __init__(self, n_nodes, tiles_per_core, f_in=F_IN, hid=HID, c_out=C_OUT,
                 k_ord=K_ORD):
        self.n = n_nodes
        self.tpc = tiles_per_core          # src tiles per core (DLOC/128)
        self.dloc = tiles_per_core * P     # nodes per core (padded)
        self.npad = self.dloc * NCORES     # padded node count
        self.nt = self.npad // P           # total src tiles
        self.f = f_in
        self.hid = hid
        self.c = c_out
        self.k = k_ord
        assert self.npad >= n_nodes
        assert f_in == P and hid == P
        # psum chunking of the dloc free dim (max 512 fp32 per bank)
        self.chunks = []
        off = 0
        while off < self.dloc:
            sz = min(512, self.dloc - off)
            self.chunks.append((off, sz))
            off += sz
        # per-chunk local tile ranges
        self.ctiles = [(off // P, (off + sz) // P) for off, sz in self.chunks]
        # chain consumption order of the 80 src tiles: all chunk-0 g tiles
        # (earliest AllGather) first, then chunk 1, then chunk 2.
        self.tile_order = [(gci, j, t)
                           for gci, (t0, t1) in enumerate(self.ctiles)
                           for j in range(NCORES)
                           for t in range(t0, t1)]
        # the last consumed src tile is pure padding (dis=0 -> g rows are
        # all zero) when its first node is >= n: skip its matmuls.
        if (self.nt - 1) * P >= n_nodes:
            self.tile_order = self.tile_order[:-1]
        # A is loaded in slabs of 8 consecutive consumption positions
        self.slab = 8
        assert self.nt % self.slab == 0
        self.n_slab = self.nt // self.slab


FULL = Geom(N, 10)  # 1280 nodes/core, npad=10240, 80 src tiles

# PE-warming filler matmuls placed at the known stall windows (start of a
# chain whose gathered g may not have landed yet) so the HAM clock gate
# stays at 8/8 across short waits.
N_WARM_START = 30
N_WARM_STEP = 12
N_WARM_FIRST = 90


def build_nc(g: Geom):
    nc = bacc.Bacc("TRN2", target_bir_lowering=False, debug=False,
                   num_devices=NCORES)

    # ---- kernel I/O ----------------------------------------------------
    # fp8 Cnt^T shard, consumption-order slabs per psum chunk:
    # [n_slab, 128, slab, chunk_sz]
    a_in = [nc.dram_tensor(f"a_in_c{ci}", [g.n_slab, P, g.slab, sz], F8E4,
                           kind="ExternalInput")
            for ci, (off, sz) in enumerate(g.chunks)]
    # initial g = dis * x, tiled [128, nt, f]
    g0_in = nc.dram_tensor("g0_in", [P, g.nt, g.f], BF16, kind="ExternalInput")
    # local x^T shard fp32 (T0 in transposed layout, for the recursion)
    xt_in = nc.dram_tensor("xt_in", [P, g.dloc], F32, kind="ExternalInput")
    # local dis*x^T bf16 (rhs of the k=0 weight term, layer 1)
    g0f_in = nc.dram_tensor("g0f_in", [P, g.dloc], BF16, kind="ExternalInput")
    # broadcast +dis rows and dinv=sqrt(deg) rows for the local shard
    disp_in = nc.dram_tensor("disp_in", [P, g.dloc], F32, kind="ExternalInput")
    dinv_in = nc.dram_tensor("dinv_in", [P, g.dloc], F32, kind="ExternalInput")
    w1_in = nc.dram_tensor("w1_in", [P, g.k, g.hid], BF16, kind="ExternalInput")
    w2_in = nc.dram_tensor("w2_in", [P, g.k, g.c], BF16, kind="ExternalInput")
    # bias columns: col 0 = b1 (hid rows), col 1 = b2 (c rows)
    bb_in = nc.dram_tensor("bb_in", [P, 2], F32, kind="ExternalInput")

    out_dram = nc.dram_tensor("out", [g.dloc, g.c], F32, kind="ExternalOutput")

    n_ag = 5  # exchanges: L1 T1, L1 T2, h, L2 T1, L2 T2

    with tile.TileContext(nc) as tc:
        with (
            tc.tile_pool(name="pers", bufs=1) as pers,
            tc.tile_pool(name="work", bufs=1) as work,
            tc.tile_pool(name="psum", bufs=1, space="PSUM") as psp,
            tc.tile_pool(name="dram", bufs=1, space="DRAM") as drp,
        ):
            # ---- persistent SBUF ---------------------------------------
            a_sb = [[pers.tile([P, g.slab, sz], F8E4, tag=f"a{ci}_{s}",
                               name=f"a{ci}_{s}")
                     for s in range(g.n_slab)]
                    for ci, (off, sz) in enumerate(g.chunks)]
            gbufC = [[pers.tile([P, NCORES, t1 - t0, g.f], BF16,
                                tag=f"g{b}_{ci}", name=f"g{b}_{ci}")
                      for ci, (t0, t1) in enumerate(g.ctiles)]
                     for b in range(2)]
            t_sb = [pers.tile([P, g.dloc], F32, tag=f"t{i}", name=f"t{i}")
                    for i in range(3)]
            disp = pers.tile([P, g.dloc], F32, name="disp")
            dinv = pers.tile([P, g.dloc], F32, name="dinv")
            gcast = pers.tile([P, g.dloc], BF16, name="gcast")
            g0f = pers.tile([P, g.dloc], BF16, name="g0f")
            hgf = pers.tile([P, g.dloc], BF16, name="hgf")
            tstage = pers.tile([P, g.tpc, g.f], BF16, name="tstage")
            w1_sb = pers.tile([P, g.k, g.hid], BF16, name="w1_sb")
            w2_sb = pers.tile([P, g.k, g.c], BF16, name="w2_sb")
            bb_sb = pers.tile([P, 2], F32, name="bb_sb")
            zf = pers.tile([g.c, g.dloc], F32, name="zf")
            idf32 = pers.tile([P, P], F32, name="idf32")
            idbf = pers.tile([P, P], BF16, name="idbf")

            # ---- DRAM bounce buffers for the collectives ---------------
            # payload row = node-in-tile (after the XBAR transpose),
            # col = (tile, feature).
            ag_srcC = [[drp.tile([P, sz], BF16, name=f"ag_src{i}_{ci}")
                        for ci, (off, sz) in enumerate(g.chunks)]
                       for i in range(n_ag)]
            ag_dstC = [[drp.tile([NCORES * P, sz], BF16,
                                 addr_space="Shared", name=f"ag_dst{i}_{ci}")
                        for ci, (off, sz) in enumerate(g.chunks)]
                       for i in range(n_ag)]
            # tiny collective to absorb comm-init + first rendezvous while
            # the inputs are still loading
            make_identity(nc, idf32[:])
            make_identity(nc, idbf[:])

            def warm(n_mm):
                """Open the PE HAM clock gate with dummy matmuls."""
                for _ in range(n_mm):
                    wp = psp.tile([P, 256], F32, space="PSUM", tag="warm",
                                  name="wp")
                    nc.tensor.matmul(wp[:, :128], lhsT=idbf[:],
                                     rhs=idbf[:], start=True, stop=True,
                                     skip_group_check=True)

            warm(N_WARM_START)

            # ---- input loads, in chain consumption order ---------------
            dges = [nc.sync, nc.scalar, nc.gpsimd]
            ld = 0

            def dma_in(dst, src):
                nonlocal ld
                dges[ld % 3].dma_start(dst, src)
                ld += 1

            dma_in(g0f[:], g0f_in[:])
            dma_in(disp[:], disp_in[:])
            dma_in(w1_sb[:], w1_in[:])
            # chunk ci of the first chain needs: its 8 g0 core-chunks and its
            # 10 A slabs, interleaved so consumption never outruns the DMA.
            for ci, (t0, t1) in enumerate(g.ctiles):
                for s in range(max(g.n_slab, NCORES)):
                    if s < NCORES:
                        j = s
                        dma_in(gbufC[0][ci][:, j, :, :],
                               g0_in[:, j * g.tpc + t0:j * g.tpc + t1, :])
                    if s < g.n_slab:
                        dma_in(a_sb[ci][s][:], a_in[ci][s])
            dma_in(t_sb[0][:], xt_in[:])
            dma_in(dinv[:], dinv_in[:])
            dma_in(w2_sb[:], w2_in[:])
            dma_in(bb_sb[:], bb_in[:])

            ag_idx = 0
            cur = 0  # g-buffer ping-pong index; gbuf[0] holds g(x)

            def stage_chunk(idx, ci, gf_src, off, sz):
                """Transpose gf chunk to node-major (PE), DMA to ag_src.
                The send path DMA lives on the sync queue only, so it can
                never head-block behind a receive-side g-load."""
                t0 = off // P
                for t in range((off) // P, (off + sz) // P):
                    tpb = psp.tile([P, P], BF16, space="PSUM", tag="tpb",
                                   name="tpb", bufs=2)
                    nc.tensor.transpose(out=tpb[:],
                                        in_=gf_src[:, t * P:(t + 1) * P],
                                        identity=idbf[:])
                    nc.vector.tensor_copy(tstage[:, t, :], tpb[:])
                nc.sync.dma_start(ag_srcC[idx][ci][:],
                                  tstage[:, t0:t0 + sz // P, :])

            gload_queue = []

            def allgather_chunk(idx, ci, b_next):
                nc.gpsimd.collective_compute(
                    "AllGather",
                    mybir.AluOpType.bypass,
                    replica_groups=[list(range(NCORES))],
                    ins=[ag_srcC[idx][ci][:]],
                    outs=[ag_dstC[idx][ci][:]],
                )
                gload_queue.append((idx, ci, b_next))

            def flush_gloads():
                """Emit the previous exchange's AG->SBUF loads. Deferred to
                the start of the consuming chain so they queue at the TAIL
                of the sync/scalar queues: nothing time-critical (stage-outs,
                AG triggers) ever head-blocks behind their AG waits."""
                gq = 0
                for idx, ci, b_next in gload_queue:
                    for j in range(NCORES):
                        eng = nc.sync if gq % 2 == 0 else nc.scalar
                        gq += 1
                        eng.dma_start(
                            gbufC[b_next][ci][:, j, :, :],
                            ag_dstC[idx][ci][j * P:(j + 1) * P, :]
                            .rearrange("p (t f) -> p t f", f=g.f),
                        )
                gload_queue.clear()

            z_all = work.tile([P, g.tpc, g.c], F32, name="z_all")
            m_all = work.tile([P, g.tpc, 1], F32, name="m_all")
            e_all = work.tile([P, g.tpc, g.c], F32, name="e_all")
            s_all = work.tile([P, g.tpc, 1], F32, name="s_all")

            def softmax_chunk(off, sz):
                t0, t1 = off // P, (off + sz) // P
                za = z_all[:, t0:t1, :]
                ma = m_all[:, t0:t1, :]
                ea = e_all[:, t0:t1, :]
                sa = s_all[:, t0:t1, :]
                shp = [P, t1 - t0, g.c]
                nc.vector.tensor_reduce(out=ma[:, :, 0], in_=za,
                                        axis=mybir.AxisListType.X,
                                        op=mybir.AluOpType.max)
                nc.vector.tensor_tensor(out=ea, in0=za,
                                        in1=ma.to_broadcast(shp),
                                        op=mybir.AluOpType.subtract)
                nc.scalar.activation(ea, ea,
                                     mybir.ActivationFunctionType.Exp)
                nc.vector.tensor_reduce(out=sa[:, :, 0], in_=ea,
                                        axis=mybir.AxisListType.X,
                                        op=mybir.AluOpType.add)
                nc.scalar.activation(sa, sa,
                                     mybir.ActivationFunctionType.Ln)
                nc.vector.tensor_add(sa, sa, ma)
                # out = z - (m + ln s), written per chunk
                nc.vector.tensor_tensor(out=ea, in0=za,
                                        in1=sa.to_broadcast(shp),
                                        op=mybir.AluOpType.subtract)
                nc.sync.dma_start(
                    out_dram.ap().rearrange("(t p) c -> p t c", p=P)
                    [:, t0:t1, :],
                    ea,
                )

            # pending PE work (weight matmuls, layer-end finalization) that
            # must trail the DVE recursion by a few chain matmuls so the
            # in-order PE never stalls on DVE results.
            pending = []

            def fire_pending():
                # High priority: the deferred W-matmuls / staging / layer-end
                # work feeds the exchange critical path; without this the
                # tile scheduler (whose internal cost model mis-estimates
                # collective and DMA-transpose latencies) slots it a full
                # chunk-chain later than emission order.
                with tc.high_priority():
                    for fn in pending:
                        fn()
                pending.clear()

            # ---- the two ChebConv layers -------------------------------
            for layer in range(2):
                w_sb = w1_sb if layer == 0 else w2_sb
                cdim = g.hid if layer == 0 else g.c
                gf0 = g0f if layer == 0 else hgf
                # persistent per-chunk PSUM accumulators for the W terms
                wt = [psp.tile([P, sz], F32, space="PSUM", tag=f"wt{ci}",
                               name=f"wt{ci}")
                      for ci, (off, sz) in enumerate(g.chunks)]

                # k=0 weight term (deferred into the T1 chain below)
                for ci, (off, sz) in enumerate(g.chunks):
                    def w0_term(ci=ci, off=off, sz=sz, wt=wt, w_sb=w_sb,
                                cdim=cdim, gf0=gf0):
                        nc.tensor.matmul(
                            wt[ci][:cdim, :sz],
                            lhsT=w_sb[:, 0, :cdim],
                            rhs=gf0[:, off:off + sz],
                            start=True, stop=False,
                        )
                    pending.append(w0_term)

                for k in range(1, g.k):
                    tk = t_sb[k % 3]
                    tk2 = t_sb[(k - 2) % 3] if k >= 2 else None
                    do_stage = k < g.k - 1  # T3 itself is never exchanged
                    do_ag = do_stage or layer == 0
                    flush_gloads()

                    for ci, (off, sz) in enumerate(g.chunks):
                        pp = psp.tile([P, 512], F32, space="PSUM", tag="pp",
                                      name="pp", bufs=2)
                        n_mm = len(g.tile_order)
                        # first chunk-chain after an exchange: its remote
                        # g-chunks can land a few µs late; short filler
                        # bursts right before the consumption points keep
                        # the HAM clock warm across those waits.
                        fresh = ci == 0 and not (layer == 0 and k == 1)
                        for n_i, (gci, j, t) in enumerate(g.tile_order):
                            lhs = gbufC[cur][gci][:, j, t - g.ctiles[gci][0], :]
                            nc.tensor.matmul(
                                pp[:, :sz],
                                lhsT=lhs,
                                rhs=a_sb[ci][n_i // g.slab][:, n_i % g.slab, :],
                                start=(n_i == 0),
                                stop=(n_i == n_mm - 1),
                            )
                            if n_i == 15:
                                fire_pending()
                            if fresh and n_mm > 64 and n_i in (31, 63):
                                warm(6 if n_i == 31 else 4)
                        # Chebyshev recursion (fp32, on DVE). High priority:
                        # this is the head of the exchange critical path.
                        with tc.high_priority():
                            nc.vector.scalar_tensor_tensor(
                                out=tk[:, off:off + sz],
                                in0=pp[:, :sz],
                                scalar=-1.0 if k == 1 else -2.0,
                                in1=disp[:, off:off + sz],
                                op0=mybir.AluOpType.mult,
                                op1=mybir.AluOpType.mult)
                            if k >= 2:
                                nc.vector.tensor_sub(
                                    tk[:, off:off + sz],
                                    tk[:, off:off + sz],
                                    tk2[:, off:off + sz])
                            # g_k = dis * T_k (bf16): chain lhsT for the
                            # next prop, staged, and W-term rhs.
                            nc.vector.tensor_tensor(
                                out=gcast[:, off:off + sz],
                                in0=tk[:, off:off + sz],
                                in1=disp[:, off:off + sz],
                                op=mybir.AluOpType.mult)
                        def w_term(ci=ci, off=off, sz=sz, k=k, wt=wt,
                                   w_sb=w_sb, cdim=cdim,
                                   do_stage=do_stage):
                            nc.tensor.matmul(
                                wt[ci][:cdim, :sz],
                                lhsT=w_sb[:, k, :cdim],
                                rhs=gcast[:, off:off + sz],
                                start=False, stop=(k == g.k - 1),
                            )
                            if do_stage:
                                stage_chunk(ag_idx, ci, gcast, off, sz)
                                allgather_chunk(ag_idx, ci, 1 - cur)
                                if ci == len(g.chunks) - 1:
                                    warm(N_WARM_FIRST if ag_idx == 0
                                         else N_WARM_STEP)
                        pending.append(w_term)

                        if k == 3 and layer == 0:
                            # layer 1 end: h = relu(dinv*wt + b1) -> t_sb[0],
                            # hgf = dis*h, stage + exchange (all off-PE).
                            def l1_end(ci=ci, off=off, sz=sz, wt=wt):
                                nc.vector.tensor_tensor(
                                    out=t_sb[0][:, off:off + sz],
                                    in0=wt[ci][:, :sz],
                                    in1=dinv[:, off:off + sz],
                                    op=mybir.AluOpType.mult)
                                nc.scalar.activation(
                                    t_sb[0][:, off:off + sz],
                                    t_sb[0][:, off:off + sz],
                                    mybir.ActivationFunctionType.Relu,
                                    bias=bb_sb[:, 0:1], scale=1.0)
                                nc.vector.tensor_tensor(
                                    out=hgf[:, off:off + sz],
                                    in0=t_sb[0][:, off:off + sz],
                                    in1=disp[:, off:off + sz],
                                    op=mybir.AluOpType.mult)
                                stage_chunk(ag_idx, ci, hgf, off, sz)
                                allgather_chunk(ag_idx, ci, 1 - cur)
                                if ci == len(g.chunks) - 1:
                                    warm(N_WARM_STEP)
                            pending.append(l1_end)

                        if k == 3 and layer == 1:
                            def l2_end(ci=ci, off=off, sz=sz, wt=wt):
                                nc.vector.tensor_tensor(
                                    out=zf[:, off:off + sz],
                                    in0=wt[ci][:g.c, :sz],
                                    in1=dinv[:g.c, off:off + sz],
                                    op=mybir.AluOpType.mult)
                                nc.scalar.activation(
                                    zf[:, off:off + sz],
                                    zf[:, off:off + sz],
                                    mybir.ActivationFunctionType.Identity,
                                    bias=bb_sb[:g.c, 1:2], scale=1.0)
                                for t in range(off // P, (off + sz) // P):
                                    zp = psp.tile([P, g.c], F32, space="PSUM",
                                                  tag="tpb", name="zp", bufs=2)
                                    nc.tensor.transpose(
                                        out=zp[:],
                                        in_=zf[:, t * P:(t + 1) * P],
                                        identity=idf32[:g.c, :g.c])
                                    nc.vector.tensor_copy(
                                        z_all[:, t, :], zp[:])
                                softmax_chunk(off, sz)
                            pending.append(l2_end)

                    fire_pending()
                    if do_ag:
                        ag_idx += 1
                        cur = 1 - cur

    nc.compile()
    return nc


def host_prep(g: Geom, x, edge_index, W1, b1, W2, b2):
    """Build the per-core input maps (sharding + dense-ification)."""
    n = g.n
    src = np.asarray(edge_index[0], dtype=np.int64)
    dst = np.asarray(edge_index[1], dtype=np.int64)
    deg = np.bincount(src, minlength=n).astype(np.float64)
    dis = np.where(deg > 0, 1.0 / np.sqrt(np.maximum(deg, 1e-12)), 0.0)
    dnv = np.where(deg > 0, np.sqrt(deg), 0.0)

    # dense-ified edge-count matrix, transposed: cnt_t[s, d]
    cnt_t = np.zeros((g.npad, g.npad), dtype=np.float32)
    np.add.at(cnt_t, (src, dst), 1.0)

    dis_pad = np.zeros(g.npad, dtype=np.float32)
    dis_pad[:n] = dis.astype(np.float32)
    dnv_pad = np.zeros(g.npad, dtype=np.float32)
    dnv_pad[:n] = dnv.astype(np.float32)
    x_pad = np.zeros((g.npad, g.f), dtype=np.float32)
    x_pad[:n] = np.asarray(x, dtype=np.float32)

    g0 = dis_pad[:, None] * x_pad  # [npad, f]
    g0_tiles = (g0.reshape(g.nt, P, g.f).transpose(1, 0, 2)
                .astype(ml_dtypes.bfloat16))  # [128, nt, f]

    w1 = np.ascontiguousarray(
        np.asarray(W1, np.float32).transpose(1, 0, 2)).astype(
            ml_dtypes.bfloat16)  # [P, k, hid]
    w2 = np.ascontiguousarray(
        np.asarray(W2, np.float32).transpose(1, 0, 2)).astype(
            ml_dtypes.bfloat16)  # [P, k, c]
    bb = np.zeros((P, 2), np.float32)
    bb[:g.hid, 0] = np.asarray(b1, np.float32)
    bb[:g.c, 1] = np.asarray(b2, np.float32)

    # chain consumption order of src tiles (must match Geom.tile_order);
    # tiles skipped by the chain (pure padding) pack at the end.
    order = [j * g.tpc + t for (gci, j, t) in g.tile_order]
    perm = np.array(order + [gi for gi in range(g.nt)
                             if gi not in set(order)])

    in_maps = []
    for c in range(NCORES):
        lo, hi = c * g.dloc, (c + 1) * g.dloc
        a_c = (cnt_t[:, lo:hi].astype(ml_dtypes.float8_e4m3)
               .reshape(g.nt, P, g.dloc))[perm]   # [nt, P, dloc] in order
        a_c = (a_c.reshape(g.n_slab, g.slab, P, g.dloc)
               .transpose(0, 2, 1, 3))            # [n_slab, P, slab, dloc]
        im = {f"a_in_c{ci}":
              np.ascontiguousarray(a_c[:, :, :, off:off + sz])
              for ci, (off, sz) in enumerate(g.chunks)}
        xt = np.ascontiguousarray(x_pad[lo:hi].T)          # [128, dloc]
        g0f = np.ascontiguousarray(
            g0[lo:hi].T.astype(ml_dtypes.bfloat16))        # [128, dloc]
        d_loc = dis_pad[lo:hi]
        disp = np.ascontiguousarray(
            np.broadcast_to(d_loc[None, :], (P, g.dloc))).astype(np.float32)
        dinv = np.ascontiguousarray(
            np.broadcast_to(dnv_pad[lo:hi][None, :],
                            (P, g.dloc))).astype(np.float32)
        im.update({
            "g0_in": np.ascontiguousarray(g0_tiles),
            "xt_in": xt,
            "g0f_in": g0f,
            "disp_in": disp,
            "dinv_in": dinv,
            "w1_in": w1,
            "w2_in": w2,
            "bb_in": bb,
        })
        in_maps.append(im)
    return in_maps


_CACHED_NC = None


def _get_nc():
    global _CACHED_NC
    if _CACHED_NC is None:
        _CACHED_NC = build_nc(FULL)
    return _CACHED_NC


def _enable_ldw_opt():
    """The default axon compile flags pass --enable-ldw-opt=false, which
    serializes every LDWEIGHTS with its MATMUL (~+107ns per matmul). Our
    kernel is a long stream of ldweights+matmul pairs, so re-enable it."""
    try:
        from concourse.compiler_utils import (get_compiler_flags,
                                              set_compiler_flags)
        flags = get_compiler_flags()
        new = [f.replace("--enable-ldw-opt=false", "--enable-ldw-opt=true")
               for f in flags]
        if new != flags:
            set_compiler_flags(new)
    except Exception:
        pass


def kernel(x, edge_index, W1, b1, W2, b2, _profile=False):
    g = FULL
    _enable_ldw_opt()
    in_maps = host_prep(g, x, edge_index, W1, b1, W2, b2)
    nc = _get_nc()
    res = run_bass_kernel_spmd(nc, in_maps, list(range(NCORES)),
                               trace=_profile)
    out = np.concatenate([res.results[c]["out"] for c in range(NCORES)], 0)
    out = out[:g.n].astype(np.float32)
    if _profile:
        kernel.last_result = res
    return out
